# revision 1
# baseline (speedup 1.0000x reference)
"""Sparse (diffusion block-causal) GQA attention on 8 Trainium2 NeuronCores.

Contract: kernel(**inputs) takes the FULL inputs
    q [2048, 4096] f32, k [2048, 1024] f32, v [2048, 1024] f32,
    block_mask [2048, 2048] bool
and returns the FULL output [2048, 4096] f32.

Sharding: tensor-parallel over KV heads. Core c owns KV head c and its 4
GQA query heads (output columns [512c, 512c+512)). block_mask handled by
compiling a per-mask-pattern schedule (full / empty / partial 128x512
tiles); partial tiles get an additive -1e30 mask folded in via an extra
accumulating identity-matmul. No inter-core communication.

Device algorithm per core (S^T layout, no on-device transposes):
  for each q-head h (4) and q-chunk J (512 wide):
    for each active k-tile j (128 wide):
      S^T[kj, qJ] = kT_j contracted with qT chunk     (PE, float32r)
      (+ -1e30 mask add via bf16 identity matmul on partial tiles,
       with fully-masked q-prefixes pruned from every matmul)
    exp via ACT (scale = 1/sqrt(128) folded in), PSUM->SBUF
    O^T[d, qJ] += V_j^T @ expS                        (PE, PSUM accum)
    softmax denominators: full tiles accumulate on DVE, one ones-vector
    matmul reduces partitions; partial tiles use ones-matmuls (PE)
  per chunk: reciprocal on DVE (custom approx op), partition-broadcast
  via a DRAM-bounce DMA, one DVE multiply normalizes, DMA out.
  Cross-chunk software pipelining: each chunk's PV group lags one exp
  group behind, epilogues overlap the next chunk's matmuls.

Host does the layout transposes during shard/gather (not part of HW time).
"""

import os
import sys

import numpy as np

for _p in ("/opt/trn_rl_repo",):
    if _p not in sys.path and os.path.isdir(_p):
        sys.path.insert(0, _p)

S = 2048
H = 32
HKV = 8
G = H // HKV  # 4 query heads per kv head
D = 128
NCORES = 8
SCALE = float(D) ** -0.5
CHUNK = 512  # q columns per S^T matmul (fp32 moving-operand max)
KT = 128  # k rows per tile (PE partition dim)
GROUP_KT = 2  # k-tiles exp'd per ACT call (2 PSUM banks)
NEG = -1.0e30

NJ = S // CHUNK  # q chunks
NK = S // KT  # k tiles

_program_cache = {}
last_exec_time_ns = None
last_results = None


def _schedule_from_mask(bm):
    """Classify each (q-chunk J, k-tile j) as full / empty / partial.

    Returns (cache_key, sched, patterns): sched[J] is a list of
    (j, pattern_idx_or_None); patterns is a list of additive-mask arrays
    [KT, CHUNK] f32 (0 where attending, NEG where masked), k-major layout
    to match the S^T tile orientation.
    """
    sched = []
    patterns = []
    pat_idx = {}
    pat_q0 = {}
    for J in range(NJ):
        rows = bm[J * CHUNK : (J + 1) * CHUNK]  # [CHUNK q, S k]
        row = []
        for j in range(NK):
            sub = rows[:, j * KT : (j + 1) * KT]  # [q, k]
            if sub.all():
                row.append((j, None, 0))
            elif not sub.any():
                continue
            else:
                key = sub.tobytes()
                if key not in pat_idx:
                    pat_idx[key] = len(patterns)
                    patterns.append(
                        np.where(sub.T, np.float32(0.0), np.float32(NEG))
                    )
                    # first q row with any active cell: columns before it
                    # are fully masked and can be skipped entirely
                    pat_q0[pat_idx[key]] = int(np.argmax(sub.any(axis=1)))
                row.append((j, pat_idx[key], pat_q0[pat_idx[key]]))
        assert row, f"q-chunk {J} attends to nothing"
        # The first tile's start=True must cover the full q range of the
        # PV/sums accumulators.
        if row[0][2] != 0:
            row[0] = (row[0][0], row[0][1], 0)
        sched.append(row)
    cache_key = tuple(
        tuple(r for r in row) for row in sched
    ), tuple(p.tobytes() for p in patterns)
    return hash(cache_key), sched, patterns


def _build_program(sched, patterns, reps=1):
    import contextlib

    import concourse.bacc as bacc
    import concourse.tile as tile
    from concourse import mybir

    f32 = mybir.dt.float32
    f32r = mybir.dt.float32r
    EXP = mybir.ActivationFunctionType.Exp
    LN = mybir.ActivationFunctionType.Ln

    nc = bacc.Bacc(
        "TRN2", target_bir_lowering=False, debug=False, num_devices=NCORES
    )

    qT = nc.dram_tensor("qT", [G, D, S], f32r, kind="ExternalInput").ap()
    kT = nc.dram_tensor("kT", [D, S], f32r, kind="ExternalInput").ap()
    v = nc.dram_tensor("v", [S, D], f32r, kind="ExternalInput").ap()
    n_pat = max(1, len(patterns))
    bf16 = mybir.dt.bfloat16
    pmask = nc.dram_tensor(
        "pmask", [n_pat, KT, CHUNK], bf16, kind="ExternalInput"
    ).ap()
    ident = nc.dram_tensor("ident", [D, D], bf16, kind="ExternalInput").ap()
    onesc = nc.dram_tensor("onesc", [KT, 1], f32r, kind="ExternalInput").ap()
    onesr = nc.dram_tensor("onesr", [1, D], f32r, kind="ExternalInput").ap()
    oT = nc.dram_tensor("oT", [G, D, S], f32, kind="ExternalOutput").ap()
    recip_d = nc.dram_tensor("recip_d", [G * NJ, CHUNK], f32).ap()

    n_chunks = G * NJ  # 16 (head, chunk) pairs

    with tile.TileContext(nc) as tc:
        with (
            tc.tile_pool(name="singles", bufs=1) as singles,
            tc.tile_pool(name="ps", bufs=2, space="PSUM") as ps_pool,
            tc.tile_pool(name="po", bufs=2, space="PSUM") as po_pool,
            tc.tile_pool(name="nrm", bufs=2, space="PSUM") as nrm_pool,
            tc.tile_pool(name="es", bufs=5) as es_pool,
            tc.tile_pool(name="otn", bufs=3) as otn_pool,
            tc.tile_pool(name="rows", bufs=4) as rows_pool,
            tc.tile_pool(name="rb", bufs=3) as rb_pool,
            tc.tile_pool(name="accp", bufs=3) as acc_pool,
        ):
            # Resident inputs. DMA order matters for the startup critical
            # path: tiny constants, then the first head/chunk's operands in
            # 512-column pieces, then the rest.
            qT_sb = singles.tile([D, G * S], f32r)
            kT_sb = singles.tile([D, S], f32r)
            v_sb = singles.tile([KT, NK * D], f32r)
            pm_sb = singles.tile([KT, n_pat * CHUNK], bf16)
            id_sb = singles.tile([D, D], bf16)
            ones_col = singles.tile([KT, 1], f32r)
            ones_colf = singles.tile([KT, 1], f32)
            nc.vector.memset(ones_colf, 1.0)
            ones_row = singles.tile([1, D], f32r)

            # Few, large input DMAs (HWDGE issue costs ~0.6us per DMA):
            # kT chunk0 + h0's first q chunk first, bulk after.
            nc.sync.dma_start(out=kT_sb[:, 0:KT], in_=kT[:, 0:KT])
            nc.sync.dma_start(
                out=qT_sb[:, 3 * CHUNK : 4 * CHUNK],
                in_=qT[0][:, 3 * CHUNK : 4 * CHUNK],
            )
            nc.sync.dma_start(out=kT_sb[:, KT:CHUNK], in_=kT[:, KT:CHUNK])
            nc.sync.dma_start(
                out=kT_sb[:, CHUNK:], in_=kT[:, CHUNK:]
            )
            nc.sync.dma_start(
                out=v_sb.rearrange("p (t d) -> p t d", d=D),
                in_=v.rearrange("(t p) d -> p t d", p=KT),
            )
            nc.sync.dma_start(
                out=pm_sb.rearrange("p (n c) -> p n c", c=CHUNK),
                in_=pmask.rearrange("n p c -> p n c"),
            )
            nc.sync.dma_start(out=ones_col, in_=onesc)
            nc.sync.dma_start(out=ones_row, in_=onesr)
            nc.sync.dma_start(out=id_sb, in_=ident)
            nc.sync.dma_start(
                out=qT_sb[:, 0 : 3 * CHUNK], in_=qT[0][:, 0 : 3 * CHUNK]
            )
            nc.sync.dma_start(
                out=qT_sb[:, S:].rearrange("p (h s) -> p h s", s=S),
                in_=qT[1:].rearrange("h p s -> p h s"),
            )

            # Staging for unnormalized O^T
            oTu = singles.tile([D, n_chunks * CHUNK], f32)

            rep_ctx = (
                tc.For_i(0, reps, 1) if reps > 1 else contextlib.nullcontext()
            )
            def emit_epilogue(ctx):
                # Normalize and store chunk ctx: runs one exp-group after
                # the chunk's last PV matmul (cross-chunk pipelined).
                cidx, h, J, po, psm = (
                    ctx["cidx"],
                    ctx["h"],
                    ctx["J"],
                    ctx["po"],
                    ctx["psm"],
                )
                otn = otn_pool.tile([D, CHUNK], f32)
                if ctx["last"]:
                    # Tail chunk: ACT reciprocal + K=1 matmul broadcast has
                    # a much shorter serial chain than the DMA bounce.
                    l_row = rows_pool.tile([1, CHUNK], f32, tag="lrow")
                    nc.scalar.activation(l_row, psm[:1, :], LN)
                    r_row = rows_pool.tile([1, CHUNK], f32r, tag="rrowr")
                    nc.scalar.activation(r_row, l_row, EXP, scale=-1.0)
                    pb = nrm_pool.tile([D, CHUNK], f32, tag="nrm")
                    nc.tensor.matmul(
                        pb, lhsT=ones_row, rhs=r_row, start=True, stop=True
                    )
                    oTu_sl = oTu[:, cidx * CHUNK : (cidx + 1) * CHUNK]
                    nc.vector.tensor_copy(oTu_sl, po)
                    nc.vector.tensor_mul(otn, oTu_sl, pb)
                else:
                    # 1/sums on DVE (single custom op, ~51 ULP) -- keeps
                    # the reciprocal off the ACT stream; broadcast across
                    # partitions via a DRAM bounce.
                    r_row = rows_pool.tile([1, CHUNK], f32, tag="rrow")
                    nc.vector.reciprocal_approx_fast(r_row, psm[:1, :])
                    nc.sync.dma_start(
                        out=recip_d[cidx : cidx + 1, :], in_=r_row
                    )
                    rb = rb_pool.tile([D, CHUNK], f32)
                    nc.sync.dma_start(
                        out=rb,
                        in_=recip_d[cidx : cidx + 1, :].partition_broadcast(
                            D
                        ),
                    )
                    oTu_sl = oTu[:, cidx * CHUNK : (cidx + 1) * CHUNK]
                    nc.vector.tensor_copy(oTu_sl, po)
                    nc.vector.tensor_mul(otn, oTu_sl, rb)
                nc.sync.dma_start(
                    out=oT[h][:, J * CHUNK : (J + 1) * CHUNK], in_=otn
                )

            def emit_pv(grp_es, grp, ctx):
                po, psm = ctx["po"], ctx["psm"]
                for t, (j, pidx, q0) in enumerate(grp):
                    sl = grp_es[:, t * CHUNK + q0 : (t + 1) * CHUNK]
                    first = ctx["pv_done"] == 0
                    last = ctx["pv_done"] == ctx["nk"] - 1
                    nc.tensor.matmul(
                        po[:, q0:],
                        lhsT=v_sb[:, j * D : (j + 1) * D],
                        rhs=sl,
                        start=first,
                        stop=last,
                    )
                    if pidx is None:
                        # Full tile: accumulate the softmax denominator
                        # contribution on DVE (keeps a third of the PE
                        # matmul streams off the critical engine).
                        slf = sl.bitcast(f32)
                        if ctx["acc"] is None:
                            ctx["acc"] = acc_pool.tile(
                                [KT, CHUNK], f32, name="acc"
                            )
                            nc.vector.tensor_copy(ctx["acc"], slf)
                        else:
                            nc.vector.tensor_add(ctx["acc"], ctx["acc"], slf)
                        ctx["nf_done"] += 1
                        if ctx["nf_done"] == ctx["nf"]:
                            # Round acc to f32r on ACT (fp32 matmuls run at
                            # 1/4 rate), then reduce over partitions.
                            accr = acc_pool.tile(
                                [KT, CHUNK], f32r, tag="accr", name="accr"
                            )
                            nc.scalar.activation(
                                accr,
                                ctx["acc"],
                                mybir.ActivationFunctionType.Copy,
                            )
                            nc.tensor.matmul(
                                psm[:1, :],
                                lhsT=ones_col,
                                rhs=accr,
                                start=True,
                                stop=(ctx["nf"] == ctx["nk"]),
                            )
                    else:
                        nc.tensor.matmul(
                            psm[:1, q0:],
                            lhsT=ones_col,
                            rhs=sl,
                            start=(ctx["nf"] == 0 and first),
                            stop=last,
                        )
                    ctx["pv_done"] += 1
                if ctx["pv_done"] == ctx["nk"]:
                    emit_epilogue(ctx)

            with rep_ctx:
                prev = None  # (es_tile, group, ctx) awaiting PV emission
                cidx = 0
                for h in range(G):
                    j_order = [3, 2, 1, 0] if (h == 0 and NJ == 4) else range(NJ)
                    for J in j_order:
                        tiles = sched[J]
                        # Full tiles exp in GROUP_KT-wide PSUM groups;
                        # partial tiles get their own unit so the exp can
                        # skip the pruned (never-written) prefix.
                        full_t = [t for t in tiles if t[1] is None]
                        part_t = [t for t in tiles if t[1] is not None]
                        ordered = full_t + part_t
                        if ordered[0][2] != 0:
                            ordered[0] = (ordered[0][0], ordered[0][1], 0)
                        groups = [
                            full_t[g : g + GROUP_KT]
                            for g in range(0, len(full_t), GROUP_KT)
                        ] + [[t] for t in ordered[len(full_t) :]]
                        ctx = {
                            "cidx": cidx,
                            "h": h,
                            "J": J,
                            "po": po_pool.tile([D, CHUNK], f32, tag="po", name="po"),
                            "psm": nrm_pool.tile(
                                [1, CHUNK], f32, tag="nrm", name="psm"
                            ),
                            "pv_done": 0,
                            "nk": len(ordered),
                            "last": (h == G - 1) and (J == NJ - 1),
                            "acc": None,
                            "nf": len(full_t),
                            "nf_done": 0,
                        }
                        rhs_q = qT_sb[
                            :, h * S + J * CHUNK : h * S + (J + 1) * CHUNK
                        ]
                        for grp in groups:
                            gw = len(grp) * CHUNK
                            lo = grp[0][2]  # >0 only for partial singleton
                            ps = ps_pool.tile(
                                [KT, len(grp) * CHUNK], f32, tag="ps"
                            )
                            for t, (j, pidx, q0) in enumerate(grp):
                                out_sl = ps[
                                    :, t * CHUNK + q0 : (t + 1) * CHUNK
                                ]
                                nc.tensor.matmul(
                                    out_sl,
                                    lhsT=kT_sb[:, j * KT : (j + 1) * KT],
                                    rhs=rhs_q[:, q0:],
                                    start=True,
                                    stop=(pidx is None),
                                )
                                if pidx is not None:
                                    nc.tensor.matmul(
                                        out_sl,
                                        lhsT=id_sb,
                                        rhs=pm_sb[
                                            :,
                                            pidx * CHUNK + q0 : (pidx + 1)
                                            * CHUNK,
                                        ],
                                        start=False,
                                        stop=True,
                                    )
                            if prev is not None:
                                emit_pv(*prev)
                                prev = None
                            es = es_pool.tile(
                                [KT, len(grp) * CHUNK], f32r, tag="es"
                            )
                            nc.scalar.activation(
                                es[:, lo:gw], ps[:, lo:gw], EXP, scale=SCALE
                            )
                            prev = (es, grp, ctx)
                        cidx += 1
                emit_pv(*prev)
                prev = None

    # Pin the ACT table set to the one containing both Exp and Ln so the
    # table-load pass emits exactly one load.
    import concourse.bacc as bacc_mod

    orig_tables = bacc_mod.get_activation_tables

    def _only_ln_exp_set(arch):
        return {
            name: (fns if name == "natural_log_exp_and_others" else set())
            for name, fns in orig_tables(arch).items()
        }

    bacc_mod.get_activation_tables = _only_ln_exp_set
    try:
        nc.compile()
    finally:
        bacc_mod.get_activation_tables = orig_tables
    return nc


def _get_program(bm):
    key, sched, patterns = _schedule_from_mask(bm)
    if key not in _program_cache:
        _program_cache[key] = _build_program(sched, patterns)
    return _program_cache[key], patterns


def _shard_inputs(q, k, v, patterns):
    import ml_dtypes

    bf16 = ml_dtypes.bfloat16
    n_pat = max(1, len(patterns))
    if patterns:
        pm = np.ascontiguousarray(np.stack(patterns).astype(bf16))
    else:
        pm = np.zeros((n_pat, KT, CHUNK), bf16)
    ident = np.eye(D, dtype=bf16)

    q5 = q.reshape(S, HKV, G, D)
    k4 = k.reshape(S, HKV, D)
    v4 = v.reshape(S, HKV, D)
    in_maps = []
    for c in range(NCORES):
        qTc = np.ascontiguousarray(q5[:, c].transpose(1, 2, 0))  # [G, D, S]
        kTc = np.ascontiguousarray(k4[:, c].T)  # [D, S]
        vc = np.ascontiguousarray(v4[:, c])  # [S, D]
        in_maps.append(
            {
                "qT": qTc,
                "kT": kTc,
                "v": vc,
                "pmask": pm,
                "ident": ident,
                "onesc": np.ones((KT, 1), np.float32),
                "onesr": np.ones((1, D), np.float32),
            }
        )
    return in_maps


def kernel(q, k, v, block_mask):
    global last_exec_time_ns, last_results
    q = np.ascontiguousarray(np.asarray(q, dtype=np.float32))
    k = np.ascontiguousarray(np.asarray(k, dtype=np.float32))
    v = np.ascontiguousarray(np.asarray(v, dtype=np.float32))
    bm = np.ascontiguousarray(np.asarray(block_mask)).astype(bool)

    nc, patterns = _get_program(bm)
    _, _, patterns = _schedule_from_mask(bm)
    in_maps = _shard_inputs(q, k, v, patterns)

    from concourse.bass_utils import run_bass_kernel_spmd

    res = run_bass_kernel_spmd(nc, in_maps, list(range(NCORES)), trace=False)
    last_exec_time_ns = res.exec_time_ns
    last_results = res

    out = np.empty((S, H * D), np.float32)
    for c in range(NCORES):
        oTc = res.results[c]["oT"]  # [G, D, S]
        out[:, c * G * D : (c + 1) * G * D] = (
            oTc.transpose(2, 0, 1).reshape(S, G * D)
        )
    return out



# revision 3
# speedup vs baseline: 1.4450x; 1.4450x over previous
"""Sparse (diffusion block-causal) GQA attention on 8 Trainium2 NeuronCores.

Contract: kernel(**inputs) takes the FULL inputs
    q [2048, 4096] f32, k [2048, 1024] f32, v [2048, 1024] f32,
    block_mask [2048, 2048] bool
and returns the FULL output [2048, 4096] f32.

Sharding: tensor-parallel over KV heads. Core c owns KV head c and its 4
GQA query heads (output columns [512c, 512c+512)). No inter-core
communication. Host does the layout transposes + bf16 casts during
shard/gather (not part of HW time).

Device algorithm per core (S^T layout [k partitions, q cols], all-bf16
matmul operands, engine-balanced):
  for each q-head h (4) and q-chunk J (512 q):
    active k-tiles (128 k rows) are classified full/partial from the
    mask; partial tiles are pruned to their live q-suffix (width
    512-q0). Pieces are packed into [128,1024] PSUM tiles (2 banks):
      S^T piece = kT_j.T @ qT chunk        (PE, bf16)
    one exp per PSUM tile (ACT, scale folded, bf16 out to SBUF).
    Partial pieces: multiplicative 0/1 staircase mask on DVE ([128,96]
    region), replacing the baseline's additive -1e30 identity-matmuls.
    PV: po[d,q] += v_j.T @ es_piece        (PE, PSUM accum)
    Denominator (partition-axis sum of es):
      partial pieces -> ones[128,128].T @ es into psm[128,512] (PE,
        broadcast across partitions for free)
      full pieces    -> bf16 adds into acc on DVE (2x rate), then one
        ones.T @ acc reduce matmul into psm
    epilogue: rb = reciprocal_approx_fast(psm) on DVE (128 lanes, same
    cost as 1), otn = po * rb (DVE), DMA out. No DRAM-bounce broadcast,
    no tail special case.
  Cross-chunk software pipelining: each PSUM tile's consumers are
  emitted one exp behind; epilogues trail into the next chunk.
"""

import os
import sys

import numpy as np

for _p in ("/opt/trn_rl_repo",):
    if _p not in sys.path and os.path.isdir(_p):
        sys.path.insert(0, _p)

S = 2048
H = 32
HKV = 8
G = H // HKV  # 4 query heads per kv head
D = 128
NCORES = 8
SCALE = float(D) ** -0.5
CHUNK = 512  # q columns per (head, J) chunk == PSUM bank (f32)
KT = 128  # k rows per tile (PE partition dim)
PS_COLS = 1024  # PSUM score tile: 2 banks

NJ = S // CHUNK  # q chunks
NK = S // KT  # k tiles

_program_cache = {}
last_exec_time_ns = None
last_results = None


def _schedule_from_mask(bm):
    """Classify each (q-chunk J, k-tile j); prune partial tiles to their
    live q-suffix (q0 rounded down to 128); pack pieces into [128,1024]
    PSUM tiles (2 x 512-col slots, no matmul write crosses a bank).

    Returns (cache_key, sched, patterns): sched[J] is a list of tiles,
    each tile a list of pieces (j, q0, w, off, pidx); patterns is a list
    of (arr [KT, CHUNK] f32 0/1 zero-padded, ext) where ext is the
    column extent that contains zeros (mask-mul region).
    """
    patterns = []
    pat_idx = {}
    sched = []
    for J in range(NJ):
        rows = bm[J * CHUNK : (J + 1) * CHUNK]  # [512 q, S k]
        partial = []
        full = []
        for j in range(NK):
            sub = rows[:, j * KT : (j + 1) * KT]  # [q, k]
            if not sub.any():
                continue
            if sub.all():
                full.append((j, 0, CHUNK, None))
                continue
            q0 = (int(np.argmax(sub.any(axis=1))) // KT) * KT
            pat = sub.T[:, q0:].astype(np.float32)  # [k, w] 0/1
            w = CHUNK - q0
            key = pat.tobytes()
            if key not in pat_idx:
                zero_cols = np.flatnonzero(~pat.all(axis=0))
                ext = int(zero_cols[-1]) + 1 if len(zero_cols) else 0
                padded = np.zeros((KT, CHUNK), np.float32)
                padded[:, :w] = pat
                pat_idx[key] = len(patterns)
                patterns.append((padded, ext))
            partial.append((j, q0, w, pat_idx[key]))
        assert partial or full, f"q-chunk {J} attends to nothing"
        # First piece must cover q [0, 512) for the PSUM start=True
        # write of both po and psm accumulation groups.
        partial.sort(key=lambda p: p[1])
        if partial:
            if partial[0][1] != 0:
                j, q0, w, pidx = partial[0]
                arr, _ = patterns[pidx]
                widened = np.zeros((KT, CHUNK), np.float32)
                widened[:, q0 : q0 + w] = arr[:, :w]
                key = widened.tobytes()
                if key not in pat_idx:
                    pat_idx[key] = len(patterns)
                    patterns.append((widened, CHUNK))  # conservative ext
                partial[0] = (j, 0, CHUNK, pat_idx[key])
        else:
            assert full[0][2] == CHUNK
        # Pack into 512-slots: partials first (q0=0 one leads), then
        # fulls. First-fit into slots.
        pieces = partial + full
        slots = []  # list of [used, [pieces...]]
        for p in pieces:
            w = p[2]
            placed = False
            for s in slots:
                if s[0] + w <= CHUNK:
                    s[1].append((p, s[0]))
                    s[0] += w
                    placed = True
                    break
            if not placed:
                slots.append([w, [(p, 0)]])
        # Pair slots into tiles; keep slot order (slot i at offset
        # 512*(i%2) in tile i//2). Put full 512-slots before partial
        # ones within a pair when it shortens the exp extent.
        tiles = []
        for t in range(0, len(slots), 2):
            pair = slots[t : t + 2]
            if len(pair) == 2 and pair[0][0] < CHUNK and pair[1][0] == CHUNK:
                pair = [pair[1], pair[0]]
            tile = []
            width = 0
            for si, (used, plist) in enumerate(pair):
                base = si * CHUNK
                for (p, off) in plist:
                    tile.append((p[0], p[1], p[2], base + off, p[3]))
                width = base + used
            tiles.append((tile, width))
        sched.append(tiles)
    cache_key = (
        tuple(
            tuple(tuple(pc) for pc in tile) + (w,)
            for row in sched
            for tile, w in row
        ),
        tuple(p[0].tobytes() for p in patterns),
    )
    return hash(cache_key), sched, patterns


def _build_program(sched, patterns, reps=1):
    import contextlib

    import concourse.bacc as bacc
    import concourse.tile as tile
    from concourse import mybir

    f32 = mybir.dt.float32
    bf16 = mybir.dt.bfloat16
    EXP = mybir.ActivationFunctionType.Exp

    nc = bacc.Bacc(
        "TRN2", target_bir_lowering=False, debug=False, num_devices=NCORES
    )

    qT = nc.dram_tensor("qT", [G, D, S], bf16, kind="ExternalInput").ap()
    kT = nc.dram_tensor("kT", [D, S], bf16, kind="ExternalInput").ap()
    v = nc.dram_tensor("v", [KT, NK * D], bf16, kind="ExternalInput").ap()
    n_pat = max(1, len(patterns))
    pmask = nc.dram_tensor(
        "pmask", [n_pat, KT, CHUNK], bf16, kind="ExternalInput"
    ).ap()
    ones = nc.dram_tensor("ones", [KT, D], bf16, kind="ExternalInput").ap()
    oT = nc.dram_tensor("oT", [G, D, S], f32, kind="ExternalOutput").ap()

    with tile.TileContext(nc) as tc:
        with (
            tc.tile_pool(name="singles", bufs=1) as singles,
            tc.tile_pool(name="ps", bufs=2, space="PSUM") as ps_pool,
            tc.tile_pool(name="po", bufs=2, space="PSUM") as po_pool,
            tc.tile_pool(name="psm", bufs=2, space="PSUM") as psm_pool,
            tc.tile_pool(name="es", bufs=4) as es_pool,
            tc.tile_pool(name="acc", bufs=2) as acc_pool,
            tc.tile_pool(name="rb", bufs=2) as rb_pool,
            tc.tile_pool(name="otn", bufs=3) as otn_pool,
        ):
            qT_sb = singles.tile([D, G * S], bf16)
            kT_sb = singles.tile([D, S], bf16)
            v_sb = singles.tile([KT, NK * D], bf16)
            pm_sb = singles.tile([KT, n_pat * CHUNK], bf16)
            ones_sb = singles.tile([KT, D], bf16)

            # DMA order = startup critical path: first k-tile + h0's
            # first-processed q chunk, then the bulk.
            nc.sync.dma_start(out=kT_sb[:, 0:KT], in_=kT[:, 0:KT])
            nc.sync.dma_start(
                out=qT_sb[:, 3 * CHUNK : 4 * CHUNK],
                in_=qT[0][:, 3 * CHUNK : 4 * CHUNK],
            )
            nc.sync.dma_start(out=kT_sb[:, KT:], in_=kT[:, KT:])
            nc.sync.dma_start(out=v_sb, in_=v)
            nc.sync.dma_start(
                out=pm_sb.rearrange("p (n c) -> p n c", c=CHUNK),
                in_=pmask.rearrange("n p c -> p n c"),
            )
            nc.sync.dma_start(out=ones_sb, in_=ones)
            nc.sync.dma_start(
                out=qT_sb[:, 0 : 3 * CHUNK], in_=qT[0][:, 0 : 3 * CHUNK]
            )
            nc.sync.dma_start(
                out=qT_sb[:, S:].rearrange("p (h s) -> p h s", s=S),
                in_=qT[1:].rearrange("h p s -> p h s"),
            )

            rep_ctx = (
                tc.For_i(0, reps, 1) if reps > 1 else contextlib.nullcontext()
            )

            def emit_epilogue(ctx):
                h, J, po, psm = ctx["h"], ctx["J"], ctx["po"], ctx["psm"]
                if ctx["acc"] is not None:
                    nc.tensor.matmul(
                        psm,
                        lhsT=ones_sb,
                        rhs=ctx["acc"],
                        start=not ctx["psm_started"],
                        stop=True,
                    )
                rb = rb_pool.tile([KT, CHUNK], f32)
                nc.vector.reciprocal_approx_fast(rb, psm)
                otn = otn_pool.tile([D, CHUNK], f32)
                nc.vector.tensor_mul(otn, po, rb)
                nc.sync.dma_start(
                    out=oT[h][:, J * CHUNK : (J + 1) * CHUNK], in_=otn
                )

            def emit_post(es, tile_pieces, ctx):
                po, psm = ctx["po"], ctx["psm"]
                for (j, q0, w, off, pidx) in tile_pieces:
                    sl = es[:, off : off + w]
                    if pidx is not None:
                        ext = patterns[pidx][1]
                        if ext:
                            nc.vector.tensor_mul(
                                es[:, off : off + ext],
                                es[:, off : off + ext],
                                pm_sb[:, pidx * CHUNK : pidx * CHUNK + ext],
                            )
                    first = ctx["pv_done"] == 0
                    last = ctx["pv_done"] == ctx["npieces"] - 1
                    nc.tensor.matmul(
                        po[:, q0:],
                        lhsT=v_sb[:, j * D : (j + 1) * D],
                        rhs=sl,
                        start=first,
                        stop=last,
                    )
                    if pidx is not None:
                        nc.tensor.matmul(
                            psm[:, q0:],
                            lhsT=ones_sb,
                            rhs=sl,
                            start=not ctx["psm_started"],
                            stop=(last and ctx["nf"] == 0),
                        )
                        ctx["psm_started"] = True
                    else:
                        if ctx["acc"] is None:
                            ctx["acc"] = acc_pool.tile(
                                [KT, CHUNK], bf16, name="acc"
                            )
                            nc.vector.tensor_copy(ctx["acc"], sl)
                        else:
                            nc.vector.tensor_add(ctx["acc"], ctx["acc"], sl)
                    ctx["pv_done"] += 1
                if ctx["pv_done"] == ctx["npieces"]:
                    emit_epilogue(ctx)

            with rep_ctx:
                prev = None
                for h in range(G):
                    j_order = (
                        [3, 2, 1, 0] if (h == 0 and NJ == 4) else range(NJ)
                    )
                    for J in j_order:
                        tiles = sched[J]
                        npieces = sum(len(t) for t, _ in tiles)
                        nf = sum(
                            1 for t, _ in tiles for p in t if p[4] is None
                        )
                        ctx = {
                            "h": h,
                            "J": J,
                            "po": po_pool.tile(
                                [D, CHUNK], f32, tag="po", name="po"
                            ),
                            "psm": psm_pool.tile(
                                [KT, CHUNK], f32, tag="psm", name="psm"
                            ),
                            "pv_done": 0,
                            "npieces": npieces,
                            "nf": nf,
                            "acc": None,
                            "psm_started": False,
                        }
                        rhs_q = qT_sb[
                            :, h * S + J * CHUNK : h * S + (J + 1) * CHUNK
                        ]
                        for (tile_pieces, width) in tiles:
                            ps = ps_pool.tile([KT, PS_COLS], f32, tag="ps")
                            for (j, q0, w, off, pidx) in tile_pieces:
                                nc.tensor.matmul(
                                    ps[:, off : off + w],
                                    lhsT=kT_sb[:, j * KT : (j + 1) * KT],
                                    rhs=rhs_q[:, q0:],
                                    start=True,
                                    stop=True,
                                )
                            if prev is not None:
                                emit_post(*prev)
                                prev = None
                            es = es_pool.tile([KT, PS_COLS], bf16, tag="es")
                            nc.scalar.activation(
                                es[:, :width], ps[:, :width], EXP, scale=SCALE
                            )
                            prev = (es, tile_pieces, ctx)
                emit_post(*prev)
                prev = None

    # Pin the ACT table set so the table-load pass emits exactly one load.
    import concourse.bacc as bacc_mod

    orig_tables = bacc_mod.get_activation_tables

    def _only_exp_set(arch):
        return {
            name: (fns if name == "natural_log_exp_and_others" else set())
            for name, fns in orig_tables(arch).items()
        }

    bacc_mod.get_activation_tables = _only_exp_set
    try:
        nc.compile()
    finally:
        bacc_mod.get_activation_tables = orig_tables
    return nc


def _get_program(bm):
    key, sched, patterns = _schedule_from_mask(bm)
    if key not in _program_cache:
        _program_cache[key] = _build_program(sched, patterns)
    return _program_cache[key], patterns


def _shard_inputs(q, k, v, patterns):
    import ml_dtypes

    bf16 = ml_dtypes.bfloat16
    n_pat = max(1, len(patterns))
    if patterns:
        pm = np.ascontiguousarray(
            np.stack([p[0] for p in patterns]).astype(bf16)
        )
    else:
        pm = np.zeros((n_pat, KT, CHUNK), bf16)
    ones = np.ones((KT, D), bf16)

    q5 = q.reshape(S, HKV, G, D)
    k4 = k.reshape(S, HKV, D)
    v4 = v.reshape(S, HKV, D)
    in_maps = []
    for c in range(NCORES):
        qTc = np.ascontiguousarray(
            q5[:, c].transpose(1, 2, 0).astype(bf16)
        )  # [G, D, S]
        kTc = np.ascontiguousarray(k4[:, c].T.astype(bf16))  # [D, S]
        vc = np.ascontiguousarray(
            v4[:, c].reshape(NK, KT, D).transpose(1, 0, 2).reshape(KT, NK * D)
            .astype(bf16)
        )  # [KT, NK*D]
        in_maps.append(
            {
                "qT": qTc,
                "kT": kTc,
                "v": vc,
                "pmask": pm,
                "ones": ones,
            }
        )
    return in_maps


def kernel(q, k, v, block_mask):
    global last_exec_time_ns, last_results
    q = np.ascontiguousarray(np.asarray(q, dtype=np.float32))
    k = np.ascontiguousarray(np.asarray(k, dtype=np.float32))
    v = np.ascontiguousarray(np.asarray(v, dtype=np.float32))
    bm = np.ascontiguousarray(np.asarray(block_mask)).astype(bool)

    nc, patterns = _get_program(bm)
    in_maps = _shard_inputs(q, k, v, patterns)

    from concourse.bass_utils import run_bass_kernel_spmd

    res = run_bass_kernel_spmd(nc, in_maps, list(range(NCORES)), trace=False)
    last_exec_time_ns = res.exec_time_ns
    last_results = res

    out = np.empty((S, H * D), np.float32)
    for c in range(NCORES):
        oTc = res.results[c]["oT"]  # [G, D, S]
        out[:, c * G * D : (c + 1) * G * D] = (
            oTc.transpose(2, 0, 1).reshape(S, G * D)
        )
    return out


# revision 16
# speedup vs baseline: 1.6637x; 1.1513x over previous
"""Sparse (diffusion block-causal) GQA attention on 8 Trainium2 NeuronCores.

Contract: kernel(**inputs) takes the FULL inputs
    q [2048, 4096] f32, k [2048, 1024] f32, v [2048, 1024] f32,
    block_mask [2048, 2048] bool
and returns the FULL output [2048, 4096] f32.

Sharding: tensor-parallel over KV heads. Core c owns KV head c and its 4
GQA query heads (output columns [512c, 512c+512)). No inter-core
communication. Host does the layout transposes + bf16 casts during
shard/gather (not part of HW time).

Device algorithm per core (S^T layout [k partitions, q cols], all-bf16
matmul operands, engine-balanced):
  for each q-head h (4) and q-chunk J (512 q):
    active k-tiles (128 k rows) are classified full/partial from the
    mask; partial tiles are pruned to their live q-suffix (width
    512-q0). Pieces are packed into [128,1024] PSUM tiles (2 banks):
      S^T piece = kT_j.T @ qT chunk        (PE, bf16)
    one exp per PSUM tile (ACT, scale folded, bf16 out to SBUF).
    Partial pieces: multiplicative 0/1 staircase mask on DVE ([128,96]
    region), replacing the baseline's additive -1e30 identity-matmuls.
    PV: po[d,q] += v_j.T @ es_piece        (PE, PSUM accum)
    Denominator (partition-axis sum of es):
      partial pieces -> ones[128,128].T @ es into psm[128,512] (PE,
        broadcast across partitions for free)
      full pieces    -> bf16 adds into acc on DVE (2x rate), then one
        ones.T @ acc reduce matmul into psm
    epilogue: rb = reciprocal_approx_fast(psm) on DVE (128 lanes, same
    cost as 1), otn = po * rb (DVE), DMA out. No DRAM-bounce broadcast,
    no tail special case.
  Cross-chunk software pipelining: each PSUM tile's consumers are
  emitted one exp behind; epilogues trail into the next chunk.
"""

import os
import sys

import numpy as np

for _p in ("/opt/trn_rl_repo",):
    if _p not in sys.path and os.path.isdir(_p):
        sys.path.insert(0, _p)

S = 2048
H = 32
HKV = 8
G = H // HKV  # 4 query heads per kv head
D = 128
NCORES = 8
SCALE = float(D) ** -0.5
CHUNK = 512  # q columns per (head, J) chunk == PSUM bank (f32)
KT = 128  # k rows per tile (PE partition dim)
PS_COLS = 1024  # PSUM score tile: 2 banks

NJ = S // CHUNK  # q chunks
NK = S // KT  # k tiles

# --- engine-balance knobs (tuned against CoreSim) ---
# Schraudolph fast-exp on DVE for the last K tiles of each chunk J
# (J=0 excluded: its 32..512-token rows need accurate exp).
SCHRAU_TILES = {}
# Epilogue po*rb multiply on GPSIMD (Pool) instead of DVE.
# (Must stay False: the Pool engine cannot read PSUM in the NEFF path.)
GPSIMD_OTN = False
# Partial-piece 0/1 mask multiply on GPSIMD instead of DVE.
GPSIMD_MASK = True
# Schraudolph constants: bf16 bits = floor(A*score + B)
SCHRAU_A = (128.0 / float(np.log(2.0))) * SCALE
SCHRAU_B = 127.0 * 128.0 - 5.5 + 0.5

_program_cache = {}
last_exec_time_ns = None
last_results = None


def _schedule_from_mask(bm):
    """Classify each (q-chunk J, k-tile j); prune partial tiles to their
    live q-suffix (q0 rounded down to 128); pack pieces into [128,1024]
    PSUM tiles (2 x 512-col slots, no matmul write crosses a bank).

    Returns (cache_key, sched, patterns): sched[J] is a list of tiles,
    each tile a list of pieces (j, q0, w, off, pidx); patterns is a list
    of (arr [KT, CHUNK] f32 0/1 zero-padded, ext) where ext is the
    column extent that contains zeros (mask-mul region).
    """
    patterns = []
    pat_idx = {}
    sched = []
    for J in range(NJ):
        rows = bm[J * CHUNK : (J + 1) * CHUNK]  # [512 q, S k]
        partial = []
        full = []
        for j in range(NK):
            sub = rows[:, j * KT : (j + 1) * KT]  # [q, k]
            if not sub.any():
                continue
            if sub.all():
                full.append((j, 0, CHUNK, None))
                continue
            q0 = (int(np.argmax(sub.any(axis=1))) // KT) * KT
            pat = sub.T[:, q0:].astype(np.float32)  # [k, w] 0/1
            w = CHUNK - q0
            key = pat.tobytes()
            if key not in pat_idx:
                zero_cols = np.flatnonzero(~pat.all(axis=0))
                ext = int(zero_cols[-1]) + 1 if len(zero_cols) else 0
                padded = np.zeros((KT, CHUNK), np.float32)
                padded[:, :w] = pat
                pat_idx[key] = len(patterns)
                patterns.append((padded, ext))
            partial.append((j, q0, w, pat_idx[key]))
        assert partial or full, f"q-chunk {J} attends to nothing"
        # First piece must cover q [0, 512) for the PSUM start=True
        # write of both po and psm accumulation groups.
        partial.sort(key=lambda p: p[1])
        if partial:
            if partial[0][1] != 0:
                j, q0, w, pidx = partial[0]
                arr, _ = patterns[pidx]
                widened = np.zeros((KT, CHUNK), np.float32)
                widened[:, q0 : q0 + w] = arr[:, :w]
                key = widened.tobytes()
                if key not in pat_idx:
                    pat_idx[key] = len(patterns)
                    patterns.append((widened, CHUNK))  # conservative ext
                partial[0] = (j, 0, CHUNK, pat_idx[key])
        else:
            assert full[0][2] == CHUNK
        # Pack into 512-slots: partials first (q0=0 one leads), then
        # fulls. First-fit into slots.
        pieces = partial + full
        slots = []  # list of [used, [pieces...]]
        for p in pieces:
            w = p[2]
            placed = False
            for s in slots:
                if s[0] + w <= CHUNK:
                    s[1].append((p, s[0]))
                    s[0] += w
                    placed = True
                    break
            if not placed:
                slots.append([w, [(p, 0)]])
        # Pair slots into tiles; keep slot order (slot i at offset
        # 512*(i%2) in tile i//2). Put full 512-slots before partial
        # ones within a pair when it shortens the exp extent.
        tiles = []
        for t in range(0, len(slots), 2):
            pair = slots[t : t + 2]
            if len(pair) == 2 and pair[0][0] < CHUNK and pair[1][0] == CHUNK:
                pair = [pair[1], pair[0]]
            tile = []
            width = 0
            for si, (used, plist) in enumerate(pair):
                base = si * CHUNK
                for (p, off) in plist:
                    tile.append((p[0], p[1], p[2], base + off, p[3]))
                width = base + used
            tiles.append((tile, width))
        sched.append(tiles)
    cache_key = (
        tuple(
            tuple(tuple(pc) for pc in tile) + (w,)
            for row in sched
            for tile, w in row
        ),
        tuple(p[0].tobytes() for p in patterns),
    )
    return hash(cache_key), sched, patterns


def _build_program(sched, patterns, reps=1):
    import contextlib

    import concourse.bacc as bacc
    import concourse.tile as tile
    from concourse import mybir

    f32 = mybir.dt.float32
    bf16 = mybir.dt.bfloat16
    EXP = mybir.ActivationFunctionType.Exp

    nc = bacc.Bacc(
        "TRN2", target_bir_lowering=False, debug=False, num_devices=NCORES
    )

    qT = nc.dram_tensor("qT", [G, D, S], bf16, kind="ExternalInput").ap()
    kT = nc.dram_tensor("kT", [D, S], bf16, kind="ExternalInput").ap()
    v = nc.dram_tensor("v", [KT, NK * D], bf16, kind="ExternalInput").ap()
    n_pat = max(1, len(patterns))
    pmask = nc.dram_tensor(
        "pmask", [n_pat, KT, CHUNK], bf16, kind="ExternalInput"
    ).ap()
    ones = nc.dram_tensor("ones", [KT, D], bf16, kind="ExternalInput").ap()
    oT = nc.dram_tensor("oT", [G, D, S], f32, kind="ExternalOutput").ap()

    with tile.TileContext(nc) as tc:
        with (
            tc.tile_pool(name="singles", bufs=1) as singles,
            tc.tile_pool(name="ps", bufs=3, space="PSUM") as ps_pool,
            tc.tile_pool(name="po", bufs=2, space="PSUM") as po_pool,
            tc.tile_pool(name="es", bufs=6) as es_pool,
            tc.tile_pool(name="acc", bufs=3) as acc_pool,
            tc.tile_pool(name="rb", bufs=3) as rb_pool,
            tc.tile_pool(name="otn", bufs=4) as otn_pool,
        ):
            qT_sb = singles.tile([D, G * S], bf16)
            kT_sb = singles.tile([D, S], bf16)
            v_sb = singles.tile([KT, NK * D], bf16)
            pm_sb = singles.tile([KT, n_pat * CHUNK], bf16)
            ones_sb = singles.tile([KT, D], bf16)

            # DMA order = startup critical path: first k-tile + h0's
            # first-processed q chunk, then the bulk.
            nc.sync.dma_start(out=kT_sb[:, 0:KT], in_=kT[:, 0:KT])
            nc.sync.dma_start(
                out=qT_sb[:, 2 * CHUNK : 4 * CHUNK],
                in_=qT[0][:, 2 * CHUNK : 4 * CHUNK],
            )
            nc.sync.dma_start(out=kT_sb[:, KT:], in_=kT[:, KT:])
            nc.sync.dma_start(out=v_sb, in_=v)
            nc.sync.dma_start(
                out=pm_sb.rearrange("p (n c) -> p n c", c=CHUNK),
                in_=pmask.rearrange("n p c -> p n c"),
            )
            nc.sync.dma_start(out=ones_sb, in_=ones)
            nc.sync.dma_start(
                out=qT_sb[:, 0 : 2 * CHUNK], in_=qT[0][:, 0 : 2 * CHUNK]
            )
            nc.sync.dma_start(
                out=qT_sb[:, S:].rearrange("p (h s) -> p h s", s=S),
                in_=qT[1:].rearrange("h p s -> p h s"),
            )

            rep_ctx = (
                tc.For_i(0, reps, 1) if reps > 1 else contextlib.nullcontext()
            )

            i16 = mybir.dt.int16
            MULT = mybir.AluOpType.mult
            ADD = mybir.AluOpType.add

            def emit_epilogue(ctx):
                h, J, po = ctx["h"], ctx["J"], ctx["po"]
                # Denominator reduce (and partition broadcast) into the
                # chunk's last score tile -- free after its exp, and the
                # 3-deep ps ring gives it ~3 tiles of lifetime.
                psm = ctx["psm_src"][:, :CHUNK]
                nc.tensor.matmul(
                    psm, lhsT=ones_sb, rhs=ctx["acc"], start=True, stop=True
                )
                rb = rb_pool.tile([KT, CHUNK], f32)
                nc.vector.reciprocal_approx_fast(rb, psm)
                otn = otn_pool.tile([D, CHUNK], f32)
                if GPSIMD_OTN:
                    nc.gpsimd.tensor_mul(otn, po, rb)
                else:
                    nc.vector.tensor_mul(otn, po, rb)
                nc.sync.dma_start(
                    out=oT[h][:, J * CHUNK : (J + 1) * CHUNK], in_=otn
                )

            def emit_post(es, tile_pieces, ctx):
                po = ctx["po"]
                for (j, q0, w, off, pidx) in tile_pieces:
                    sl = es[:, off : off + w]
                    if pidx is not None:
                        ext = patterns[pidx][1]
                        if ext:
                            eng = nc.gpsimd if GPSIMD_MASK else nc.vector
                            eng.tensor_mul(
                                es[:, off : off + ext],
                                es[:, off : off + ext],
                                pm_sb[:, pidx * CHUNK : pidx * CHUNK + ext],
                            )
                    first = ctx["pv_done"] == 0
                    last = ctx["pv_done"] == ctx["npieces"] - 1
                    nc.tensor.matmul(
                        po[:, q0:],
                        lhsT=v_sb[:, j * D : (j + 1) * D],
                        rhs=sl,
                        start=first,
                        stop=last,
                    )
                    if ctx["acc"] is None:
                        assert q0 == 0, "first acc piece must cover q0=0"
                        ctx["acc"] = acc_pool.tile(
                            [KT, CHUNK], bf16, name="acc"
                        )
                        nc.vector.tensor_copy(ctx["acc"], sl)
                    else:
                        nc.vector.tensor_add(
                            ctx["acc"][:, q0:], ctx["acc"][:, q0:], sl
                        )
                    ctx["pv_done"] += 1
                if ctx["pv_done"] == ctx["npieces"]:
                    emit_epilogue(ctx)

            def make_chunk(h, J):
                tiles = sched[J]
                npieces = sum(len(t) for t, _ in tiles)
                n_schrau = SCHRAU_TILES.get(J, 0)
                ctx = {
                    "h": h,
                    "J": J,
                    "po": po_pool.tile([D, CHUNK], f32, tag="po", name="po"),
                    "pv_done": 0,
                    "npieces": npieces,
                    "acc": None,
                    "psm_src": None,
                }
                return {
                    "ctx": ctx,
                    "tiles": tiles,
                    "t_i": 0,
                    "n_schrau": n_schrau,
                    "rhs_q": qT_sb[
                        :, h * S + J * CHUNK : h * S + (J + 1) * CHUNK
                    ],
                }

            def emit_tile(st):
                """Emit one PSUM tile of chunk-state st: S^T matmuls,
                then (pipelined) post-ops of the globally previous tile,
                then this tile's exp."""
                tiles, t_i, ctx = st["tiles"], st["t_i"], st["ctx"]
                tile_pieces, width = tiles[t_i]
                ps = ps_pool.tile([KT, PS_COLS], f32, tag="ps")
                for (j, q0, w, off, pidx) in tile_pieces:
                    nc.tensor.matmul(
                        ps[:, off : off + w],
                        lhsT=kT_sb[:, j * KT : (j + 1) * KT],
                        rhs=st["rhs_q"][:, q0:],
                        start=True,
                        stop=True,
                    )
                if t_i == len(tiles) - 1:
                    ctx["psm_src"] = ps
                if pending and pending[0][0] == "schrau":
                    _, p_ps, p_es, p_w, p_pieces, p_ctx = pending[0]
                    # Deferred Schraudolph: by now its S^T matmuls have
                    # retired, so this never stalls the DVE queue.
                    nc.vector.tensor_scalar(
                        p_es[:, :p_w].bitcast(i16),
                        p_ps[:, :p_w],
                        SCHRAU_A,
                        SCHRAU_B,
                        MULT,
                        ADD,
                    )
                    pending[0] = ("post", p_es, p_pieces, p_ctx)
                if pending:
                    pend = pending.pop(0)
                    emit_post(pend[1], pend[2], pend[3])
                es = es_pool.tile([KT, PS_COLS], bf16, tag="es")
                if t_i >= len(tiles) - st["n_schrau"]:
                    pending.append(("schrau", ps, es, width, tile_pieces, ctx))
                else:
                    nc.scalar.activation(
                        es[:, :width], ps[:, :width], EXP, scale=SCALE
                    )
                    pending.append(("post", es, tile_pieces, ctx))
                st["t_i"] += 1
                return st["t_i"] < len(st["tiles"])

            with rep_ctx:
                pending = []
                # Rolling 2-deep chunk window: interleave two chunks'
                # tiles so each chunk's S->exp->PV chain has 2-tile
                # spacing and semaphore latencies hide behind the other
                # chunk's work.
                order = []
                for h in range(G):
                    j_order = (
                        [3, 2, 1, 0] if (h == 0 and NJ == 4) else range(NJ)
                    )
                    for J in j_order:
                        order.append((h, J))
                queue = list(order)
                active = []
                while queue or active:
                    while len(active) < 2 and queue:
                        h, J = queue.pop(0)
                        active.append(make_chunk(h, J))
                    st = active.pop(0)
                    if emit_tile(st):
                        active.append(st)
                while pending:
                    pend = pending.pop(0)
                    if pend[0] == "schrau":
                        _, p_ps, p_es, p_w, p_pieces, p_ctx = pend
                        nc.vector.tensor_scalar(
                            p_es[:, :p_w].bitcast(i16),
                            p_ps[:, :p_w],
                            SCHRAU_A,
                            SCHRAU_B,
                            MULT,
                            ADD,
                        )
                        emit_post(p_es, p_pieces, p_ctx)
                    else:
                        emit_post(pend[1], pend[2], pend[3])

    # Pin the ACT table set so the table-load pass emits exactly one load.
    import concourse.bacc as bacc_mod

    orig_tables = bacc_mod.get_activation_tables

    def _only_exp_set(arch):
        return {
            name: (fns if name == "natural_log_exp_and_others" else set())
            for name, fns in orig_tables(arch).items()
        }

    bacc_mod.get_activation_tables = _only_exp_set
    try:
        nc.compile()
    finally:
        bacc_mod.get_activation_tables = orig_tables
    return nc


def _get_program(bm):
    key, sched, patterns = _schedule_from_mask(bm)
    if key not in _program_cache:
        _program_cache[key] = _build_program(sched, patterns)
    return _program_cache[key], patterns


def _shard_inputs(q, k, v, patterns):
    import ml_dtypes

    bf16 = ml_dtypes.bfloat16
    n_pat = max(1, len(patterns))
    if patterns:
        pm = np.ascontiguousarray(
            np.stack([p[0] for p in patterns]).astype(bf16)
        )
    else:
        pm = np.zeros((n_pat, KT, CHUNK), bf16)
    ones = np.ones((KT, D), bf16)

    q5 = q.reshape(S, HKV, G, D)
    k4 = k.reshape(S, HKV, D)
    v4 = v.reshape(S, HKV, D)
    in_maps = []
    for c in range(NCORES):
        qTc = np.ascontiguousarray(
            q5[:, c].transpose(1, 2, 0).astype(bf16)
        )  # [G, D, S]
        kTc = np.ascontiguousarray(k4[:, c].T.astype(bf16))  # [D, S]
        vc = np.ascontiguousarray(
            v4[:, c].reshape(NK, KT, D).transpose(1, 0, 2).reshape(KT, NK * D)
            .astype(bf16)
        )  # [KT, NK*D]
        in_maps.append(
            {
                "qT": qTc,
                "kT": kTc,
                "v": vc,
                "pmask": pm,
                "ones": ones,
            }
        )
    return in_maps


def kernel(q, k, v, block_mask):
    global last_exec_time_ns, last_results
    q = np.ascontiguousarray(np.asarray(q, dtype=np.float32))
    k = np.ascontiguousarray(np.asarray(k, dtype=np.float32))
    v = np.ascontiguousarray(np.asarray(v, dtype=np.float32))
    bm = np.ascontiguousarray(np.asarray(block_mask)).astype(bool)

    nc, patterns = _get_program(bm)
    in_maps = _shard_inputs(q, k, v, patterns)

    from concourse.bass_utils import run_bass_kernel_spmd

    res = run_bass_kernel_spmd(nc, in_maps, list(range(NCORES)), trace=False)
    last_exec_time_ns = res.exec_time_ns
    last_results = res

    out = np.empty((S, H * D), np.float32)
    for c in range(NCORES):
        oTc = res.results[c]["oT"]  # [G, D, S]
        out[:, c * G * D : (c + 1) * G * D] = (
            oTc.transpose(2, 0, 1).reshape(S, G * D)
        )
    return out
